# revision 11
# baseline (speedup 1.0000x reference)
"""CFR_flow_t_align (DeMFI) forward-warp kernel for 8x Trainium2 NeuronCores.

Strategy (v2 rewrite)
---------------------
Pure data-parallel over batch N: core i processes image i (the scatter-add's
flat index space never crosses images; no collectives).

Per warp the splat is computed as a dense masked accumulation over integer
displacement buckets (A = row tap, B = col tap):

    rowpsi_A = (afl==A)*wr1 + (afl==A-1)*wr2          (exact reference taps)
    psic_B   = (bfl==B)*wc1 + (bfl==B-1)*wc2
    dacc_A[:, w+B] += (v3 * rowpsi_A) * psic_B        (per occupied (A,B))

Column shifts are free-dim AP offsets; row shifts are SBUF->SBUF DMA
partition rotations of dacc_A into a canvas.  Occupied (A,B) sets are
derived on the host per core/band (masks make any superset correct).

Perf-critical choices vs the old baseline:
  * per-core work is guarded once per (warp, band) instead of per (A, core),
    removing thousands of tiny If blocks;
  * all hot ops are bf16 and kept in the DVE 2x perf mode: pair-accumulators
    are split by column-offset parity (daccE holds true columns, daccO holds
    columns shifted by one) so every tensor_tensor offset is 4B-aligned;
    the two canvases are merged once per warp with a single 1x op;
  * the first pair of each (A, band, parity) writes dacc directly (no
    accumulator zeroing; only a tiny right-margin memset);
  * Gaussian tap weights come from the Scalar engine (Square/Exp), floors
    from the BIGC round trick (exact for |x| << 2^22).
"""

import math

import numpy as np

P = 128
BIGC = 1.5 * float(1 << 23)
W2 = 1024  # dacc/canvas padded width (PSUM-friendly, fixed)


# ---------------------------------------------------------------------------
# Host-side plan derivation (sizing/occupancy only -- all math runs on device)
# ---------------------------------------------------------------------------

def _derive_plan(flow_01, flow_10, t_value):
    n, _, H, W = flow_01.shape
    t = np.asarray(t_value, dtype=np.float32).reshape(n)
    NB = (H + P - 1) // P

    plan = []  # plan[core][warp] = per-band structure
    a_min = b_min = 10 ** 9
    a_max = b_max = -10 ** 9
    nB_max = 0
    for core in range(n):
        warps = []
        for w in range(2):
            s = np.float32(t[core]) if w == 0 else np.float32(1.0) - np.float32(t[core])
            flow = np.asarray(flow_01[core] if w == 0 else flow_10[core], np.float32)
            xs = np.float32(s) * flow[1]
            ys = np.float32(s) * flow[0]
            afl = np.floor(xs).astype(np.int64)
            bfl = np.floor(ys).astype(np.int64)
            bands = []
            for b in range(NB):
                a = afl[b * P:(b + 1) * P].ravel()
                bb = bfl[b * P:(b + 1) * P].ravel()
                keys = np.unique((a + 64) * 256 + (bb + 64))
                cells = set()
                for k in keys:
                    A, B = int(k // 256) - 64, int(k % 256) - 64
                    for da in (0, 1):
                        for db in (0, 1):
                            cells.add((A + da, B + db))
                byA = {}
                for (A, B) in cells:
                    byA.setdefault(A, {0: [], 1: []})[B & 1].append(B)
                    a_min = min(a_min, A); a_max = max(a_max, A)
                    b_min = min(b_min, B); b_max = max(b_max, B)
                blist = sorted({B for (_, B) in cells})
                nB_max = max(nB_max, len(blist))
                bands.append({
                    "A": {A: {p: sorted(v) for p, v in d.items() if v} for A, d in sorted(byA.items())},
                    "B": blist,
                })
            warps.append(bands)
        plan.append(warps)

    marg = 16
    while marg + b_min - 1 < 0:
        marg += 2
    assert marg + b_max + W <= W2, (marg, b_max)
    row_off = 32 * math.ceil(max(0, -a_min) / 32)
    HC = (row_off + H + a_max + 1 + 127) // 128 * 128
    geom = dict(marg=marg, row_off=row_off, HC=HC, HCB=HC // 128, NB=NB,
                H=H, W=W, nB_max=max(nB_max, 1))
    return plan, geom


# ---------------------------------------------------------------------------
# Device program
# ---------------------------------------------------------------------------

def _build_program(plan, geom, n_cores):
    import concourse.bacc as bacc
    import concourse.mybir as mybir
    import concourse.tile as tile

    f32 = mybir.dt.float32
    bf16 = mybir.dt.bfloat16
    Alu = mybir.AluOpType
    Act = mybir.ActivationFunctionType

    H, W = geom["H"], geom["W"]
    MARG, ROW_OFF = geom["marg"], geom["row_off"]
    HC, HCB, NB = geom["HC"], geom["HCB"], geom["NB"]
    nB_max = geom["nB_max"]
    band_rows = [min(P, H - P * b) for b in range(NB)]

    nc = bacc.Bacc("TRN2", enable_partition_id=True)
    d_f01 = nc.dram_tensor("flow01", [2, H, W], f32, kind="ExternalInput")
    d_f10 = nc.dram_tensor("flow10", [2, H, W], f32, kind="ExternalInput")
    d_tv = nc.dram_tensor("tv", [P, 1], f32, kind="ExternalInput")
    d_out0 = nc.dram_tensor("out0", [2, H, W], f32, kind="ExternalOutput")
    d_out1 = nc.dram_tensor("out1", [2, H, W], f32, kind="ExternalOutput")

    with tile.TileContext(nc) as tc:
        with (
            tc.tile_pool(name="dram", bufs=1, space="DRAM") as dram_pool,
            tc.tile_pool(name="const", bufs=1) as const_pool,
            tc.tile_pool(name="canvas", bufs=1) as canvas_pool,
            tc.tile_pool(name="band32", bufs=1) as band_pool,      # f32 planes
            tc.tile_pool(name="band16", bufs=1) as band16_pool,    # bf16 planes
            tc.tile_pool(name="cache", bufs=1) as cache_pool,
            tc.tile_pool(name="rowg", bufs=1) as rowg_pool,
            tc.tile_pool(name="dacc", bufs=1) as dacc_pool,
            tc.tile_pool(name="scr", bufs=1) as scr_pool,
        ):
            comb_pool = band_pool  # combine temps reuse the [P, W] f32 slots
            c0_hbm = dram_pool.tile([HC, 3, W2], bf16)

            # ---- scalars (tv arrives replicated across partitions) -------
            t_sb = const_pool.tile([P, 1], f32)
            nc.sync.dma_start(out=t_sb[:, :], in_=d_tv[:, :])
            omt = const_pool.tile([P, 1], f32)
            nc.vector.tensor_scalar(out=omt[:, :], in0=t_sb[:, :], scalar1=-1.0,
                                    scalar2=1.0, op0=Alu.mult, op1=Alu.add)
            al0 = const_pool.tile([P, 1], f32)   # -(1-t)*t
            nc.vector.tensor_tensor(out=al0[:, :], in0=omt[:, :], in1=t_sb[:, :], op=Alu.mult)
            nc.vector.tensor_scalar(out=al0[:, :], in0=al0[:, :], scalar1=-1.0, scalar2=None, op0=Alu.mult)
            al1 = const_pool.tile([P, 1], f32)   # t^2
            nc.vector.tensor_tensor(out=al1[:, :], in0=t_sb[:, :], in1=t_sb[:, :], op=Alu.mult)
            be0 = const_pool.tile([P, 1], f32)   # (1-t)^2
            nc.vector.tensor_tensor(out=be0[:, :], in0=omt[:, :], in1=omt[:, :], op=Alu.mult)
            neg1 = const_pool.tile([P, 1], f32)
            nc.vector.memset(neg1[:, :], -1.0)

            pid = nc.vector.partition_id()
            state = {}

            def bc3(ap2d):
                return ap2d.rearrange("p (o w) -> p o w", o=1).to_broadcast([P, 3, W])

            def part_windows(lo, hi):
                # naturally-aligned partition blocks covering [lo, hi)
                out = []
                while lo < hi:
                    sz = 128
                    while sz > 32 and (lo % sz != 0 or lo + sz > hi):
                        sz //= 2
                    out.append((lo, sz))
                    lo += sz
                return out

            def zero_pad_rows(t3, rows, val=0.0):
                p_ = rows
                while p_ < P:
                    ln = {0: P, 32: 32, 64: 64, 96: 32}[p_]
                    nc.vector.memset(t3[p_:p_ + ln], val)
                    p_ += ln

            def do_warp(warp, flow_dram, s_ap):
                canvE = canvas_pool.tile([P, HCB, 3, W2], bf16, tag="canvE")
                canvO = canvas_pool.tile([P, HCB, 3, W2], bf16, tag="canvO")
                state["canv"] = canvE
                nc.vector.memset(canvE[:, :, :, :], 0.0)
                nc.vector.memset(canvO[:, :, :, :], 0.0)

                for b in range(NB):
                    rows = band_rows[b]

                    xs = band_pool.tile([P, W], f32, tag="xs")
                    ys = band_pool.tile([P, W], f32, tag="ys")
                    nc.sync.dma_start(out=ys[0:rows, :], in_=flow_dram[0, P * b:P * b + rows, :])
                    nc.sync.dma_start(out=xs[0:rows, :], in_=flow_dram[1, P * b:P * b + rows, :])
                    if rows < P:
                        zero_pad_rows(xs, rows)
                        zero_pad_rows(ys, rows)

                    v3h = band16_pool.tile([P, 3, W], bf16, tag="v3h")
                    nc.vector.tensor_copy(v3h[:, 0, :], ys[:, :])
                    nc.vector.tensor_copy(v3h[:, 1, :], xs[:, :])
                    nc.vector.memset(v3h[0:P if rows == P else rows, 2, :], 1.0)
                    if rows < P:
                        zero_pad_rows(v3h[:, 2, :], rows)

                    nc.vector.tensor_scalar(out=xs[:, :], in0=xs[:, :], scalar1=s_ap, scalar2=None, op0=Alu.mult)
                    nc.vector.tensor_scalar(out=ys[:, :], in0=ys[:, :], scalar1=s_ap, scalar2=None, op0=Alu.mult)

                    # floors (BIGC round trick) + fractional parts
                    def floor_frac(src, flh_t, w1_t, w2_t):
                        r = band_pool.tile([P, W], f32, tag="r")
                        nc.vector.tensor_scalar(out=r[:, :], in0=src[:, :], scalar1=BIGC,
                                                scalar2=BIGC, op0=Alu.add, op1=Alu.subtract)
                        m = band_pool.tile([P, W], f32, tag="m")
                        nc.vector.tensor_tensor(out=m[:, :], in0=r[:, :], in1=src[:, :], op=Alu.is_gt)
                        fl = band_pool.tile([P, W], f32, tag="fl")
                        nc.vector.tensor_tensor(out=fl[:, :], in0=r[:, :], in1=m[:, :], op=Alu.subtract)
                        nc.vector.tensor_copy(flh_t[:, :], fl[:, :])
                        fx = r
                        nc.vector.tensor_tensor(out=fx[:, :], in0=src[:, :], in1=fl[:, :], op=Alu.subtract)
                        sq = m
                        nc.scalar.activation(sq[:, :], fx[:, :], Act.Square)
                        nc.scalar.activation(w1_t[:, :], sq[:, :], Act.Exp, scale=-1.0)
                        nc.scalar.activation(sq[:, :], fx[:, :], Act.Square, bias=neg1[:, 0:1])
                        nc.scalar.activation(w2_t[:, :], sq[:, :], Act.Exp, scale=-1.0)

                    aflh = band16_pool.tile([P, W], bf16, tag="aflh")
                    bflh = band16_pool.tile([P, W], bf16, tag="bflh")
                    wr1 = band16_pool.tile([P, W], bf16, tag="wr1")
                    wr2 = band16_pool.tile([P, W], bf16, tag="wr2")
                    wc1 = band16_pool.tile([P, W], bf16, tag="wc1")
                    wc2 = band16_pool.tile([P, W], bf16, tag="wc2")
                    floor_frac(xs, aflh, wr1, wr2)
                    floor_frac(ys, bflh, wc1, wc2)

                    # tiles used inside the per-core guards are allocated
                    # outside them (a skipped branch must not own pool slots)
                    psic = cache_pool.tile([P, nB_max, W], bf16, tag="psic")
                    tpa = band16_pool.tile([P, W], bf16, tag="tpa")
                    m1 = band16_pool.tile([P, W], bf16, tag="m1")
                    mc = band16_pool.tile([P, W], bf16, tag="mc")
                    rowg = rowg_pool.tile([P, 3, W], bf16, tag="rowg")
                    tmp3 = rowg_pool.tile([P, 3, W], bf16, tag="tmp3")
                    daccE_t = dacc_pool.tile([P, 3, W2], bf16, tag="daccE")
                    daccO_t = dacc_pool.tile([P, 3, W2], bf16, tag="daccO")
                    daccs = [daccE_t, daccO_t]
                    scrA_t = scr_pool.tile([P, 3, W2], bf16, tag="scrA")
                    scrB_t = scr_pool.tile([P, 3, W2], bf16, tag="scrB")
                    scrs = [scrA_t, scrB_t]
                    scr_i = 0
                    # --- per-core psic caches (DVE-only guarded bodies) ---
                    for ci in range(n_cores):
                        bp = plan[ci][warp][b]
                        if not bp["B"]:
                            continue
                        with tc.If(pid == ci):
                            bidx = {B: j for j, B in enumerate(bp["B"])}
                            for B in bp["B"]:
                                pj = psic[:, bidx[B], :]
                                nc.vector.scalar_tensor_tensor(
                                    out=pj, in0=bflh[:, :], scalar=float(B), in1=wc1[:, :],
                                    op0=Alu.is_equal, op1=Alu.mult)
                                nc.vector.scalar_tensor_tensor(
                                    out=tpa[:, :], in0=bflh[:, :], scalar=float(B - 1), in1=wc2[:, :],
                                    op0=Alu.is_equal, op1=Alu.mult)
                                nc.vector.tensor_tensor(out=pj, in0=pj, in1=tpa[:, :], op=Alu.add)

                    # --- union instance loop: guarded DVE writes into dacc,
                    # unconditional zero / rotate-DMA / canvas-add (no DMA or
                    # pool traffic may live inside a branch) ---
                    union_A = sorted({A for ci in range(n_cores)
                                      for A in plan[ci][warp][b]["A"]})
                    for A in union_A:
                        members = [ci for ci in range(n_cores)
                                   if A in plan[ci][warp][b]["A"]]
                        hulls = {}
                        for par in (0, 1):
                            los = [MARG + plan[ci][warp][b]["A"][A][par][0] - par
                                   for ci in members if par in plan[ci][warp][b]["A"][A]]
                            his = [MARG + plan[ci][warp][b]["A"][A][par][-1] - par + W
                                   for ci in members if par in plan[ci][warp][b]["A"][A]]
                            if los:
                                hulls[par] = (min(los), max(his))
                        for par, (h0, h1) in hulls.items():
                            nc.vector.memset(daccs[par][:, :, h0:h1], 0.0)
                        for ci in members:
                            pdict = plan[ci][warp][b]["A"][A]
                            bidx = {B: j for j, B in enumerate(plan[ci][warp][b]["B"])}
                            with tc.If(pid == ci):
                                nc.vector.scalar_tensor_tensor(
                                    out=m1[:, :], in0=aflh[:, :], scalar=float(A), in1=wr1[:, :],
                                    op0=Alu.is_equal, op1=Alu.mult)
                                nc.vector.scalar_tensor_tensor(
                                    out=mc[:, :], in0=aflh[:, :], scalar=float(A - 1), in1=wr2[:, :],
                                    op0=Alu.is_equal, op1=Alu.mult)
                                nc.vector.tensor_tensor(out=mc[:, :], in0=mc[:, :], in1=m1[:, :], op=Alu.add)
                                nc.vector.tensor_tensor(out=rowg[:, :, :], in0=bc3(mc[:, :]),
                                                        in1=v3h[:, :, :], op=Alu.mult)
                                for par, bl in pdict.items():
                                    # dacc element k represents column (k - MARG + par)
                                    dacc = daccs[par]
                                    for B in bl:
                                        psi = bc3(psic[:, bidx[B], :])
                                        o = MARG + B - par
                                        nc.vector.tensor_tensor(out=tmp3[:, :, :], in0=rowg[:, :, :],
                                                                in1=psi, op=Alu.mult)
                                        dst = dacc[:, :, o:o + W]
                                        nc.vector.tensor_tensor(out=dst, in0=dst, in1=tmp3[:, :, :], op=Alu.add)

                        # rotate rows by A into the canvases (unconditional)
                        s0 = P * b + A + ROW_OFF
                        jlo, p0 = divmod(s0, P)
                        len1 = min(rows, P - p0)
                        pieces = [(jlo, p0, 0, len1)]
                        if len1 < rows:
                            pieces.append((jlo + 1, 0, len1, rows - len1))
                        for par, (h0, h1) in hulls.items():
                            canv = canvE if par == 0 else canvO
                            dacc = daccs[par]
                            for (jb, q0, src0, ln) in pieces:
                                scr = scrs[scr_i % 2]; scr_i += 1
                                w_lo = (q0 // 32) * 32
                                w_hi = min(P, ((q0 + ln + 31) // 32) * 32)
                                # edge 32-blocks zeroed first, DMA overwrites interior
                                if q0 > w_lo:
                                    nc.vector.memset(scr[w_lo:w_lo + 32, :, h0:h1], 0.0)
                                if w_hi > q0 + ln and (w_hi - 32 > w_lo or q0 == w_lo):
                                    nc.vector.memset(scr[w_hi - 32:w_hi, :, h0:h1], 0.0)
                                nc.sync.dma_start(out=scr[q0:q0 + ln, :, h0:h1],
                                                  in_=dacc[src0:src0 + ln, :, h0:h1])
                                for (plo, psz) in part_windows(w_lo, w_hi):
                                    dstc = canv[plo:plo + psz, jb, :, h0:h1]
                                    nc.vector.tensor_tensor(
                                        out=dstc, in0=dstc,
                                        in1=scr[plo:plo + psz, :, h0:h1], op=Alu.add)

                # merge odd canvas into even (single 1x op per warp)
                flatE = canvE.rearrange("p h c w -> p (h c w)")
                flatO = canvO.rearrange("p h c w -> p (h c w)")
                ncols = HCB * 3 * W2
                nc.vector.tensor_tensor(out=flatE[:, 1:ncols], in0=flatE[:, 1:ncols],
                                        in1=flatO[:, 0:ncols - 1], op=Alu.add)

            # ---- warp 0: img=flow01, shift=t*flow01 ---------------------
            do_warp(0, d_f01, t_sb[:, 0:1])
            canvas0 = state["canv"]
            for jb in range(HCB):
                nc.sync.dma_start(out=c0_hbm[P * jb:P * jb + P, :, :], in_=canvas0[:, jb, :, :])

            # ---- warp 1: img=flow10, shift=(1-t)*flow10 -----------------
            do_warp(1, d_f10, omt[:, 0:1])
            canvas1 = state["canv"]

            # ---- combine -------------------------------------------------
            sl = slice(MARG, MARG + W)
            for jb in range(HCB):
                lo = P * jb
                o_lo = max(0, lo - ROW_OFF)
                o_hi = min(H, lo + P - ROW_OFF)
                if o_lo >= o_hi:
                    continue
                cv0 = scr_pool.tile([P, 3, W2], bf16, tag="scrA")
                nc.sync.dma_start(out=cv0[:, :, :], in_=c0_hbm[lo:lo + P, :, :])

                tn1 = comb_pool.tile([P, W], f32, tag="xs")
                nc.vector.tensor_scalar(out=tn1[:, :], in0=canvas1[:, jb, 2, sl],
                                        scalar1=t_sb[:, 0:1], scalar2=1.0,
                                        op0=Alu.mult, op1=Alu.add)
                nhat = comb_pool.tile([P, W], f32, tag="ys")
                nc.vector.scalar_tensor_tensor(
                    out=nhat[:, :], in0=cv0[:, 2, sl], scalar=omt[:, 0:1], in1=tn1[:, :],
                    op0=Alu.mult, op1=Alu.add)
                mgt = comb_pool.tile([P, W], f32, tag="r")
                nc.vector.tensor_scalar(out=mgt[:, :], in0=nhat[:, :], scalar1=1.0, scalar2=None, op0=Alu.is_gt)
                den = comb_pool.tile([P, W], f32, tag="m")
                nc.vector.tensor_tensor(out=den[:, :], in0=nhat[:, :], in1=mgt[:, :], op=Alu.subtract)
                rec = comb_pool.tile([P, W], f32, tag="fl")
                nc.vector.reciprocal(rec[:, :], den[:, :])

                p_lo = o_lo + ROW_OFF - lo
                p_hi = o_hi + ROW_OFF - lo
                for c in range(2):
                    u = comb_pool.tile([P, W], f32, tag="xs")
                    o0 = comb_pool.tile([P, W], f32, tag="o0")
                    o1 = comb_pool.tile([P, W], f32, tag="o1")
                    nc.vector.tensor_scalar(out=u[:, :], in0=cv0[:, c, sl],
                                            scalar1=al0[:, 0:1], scalar2=None, op0=Alu.mult)
                    nc.vector.scalar_tensor_tensor(
                        out=o0[:, :], in0=canvas1[:, jb, c, sl], scalar=al1[:, 0:1], in1=u[:, :],
                        op0=Alu.mult, op1=Alu.add)
                    nc.vector.tensor_tensor(out=o0[:, :], in0=o0[:, :], in1=rec[:, :], op=Alu.mult)
                    nc.vector.tensor_scalar(out=u[:, :], in0=cv0[:, c, sl],
                                            scalar1=be0[:, 0:1], scalar2=None, op0=Alu.mult)
                    nc.vector.scalar_tensor_tensor(
                        out=o1[:, :], in0=canvas1[:, jb, c, sl], scalar=al0[:, 0:1], in1=u[:, :],
                        op0=Alu.mult, op1=Alu.add)
                    nc.vector.tensor_tensor(out=o1[:, :], in0=o1[:, :], in1=rec[:, :], op=Alu.mult)
                    nc.sync.dma_start(out=d_out0[c, o_lo:o_hi, :], in_=o0[p_lo:p_hi, :])
                    nc.sync.dma_start(out=d_out1[c, o_lo:o_hi, :], in_=o1[p_lo:p_hi, :])

    nc.finalize()
    return nc


# ---------------------------------------------------------------------------
# Entry point
# ---------------------------------------------------------------------------

def _prepare(flow_01, flow_10, t_value):
    flow_01 = np.ascontiguousarray(np.asarray(flow_01, dtype=np.float32))
    flow_10 = np.ascontiguousarray(np.asarray(flow_10, dtype=np.float32))
    t_value = np.ascontiguousarray(np.asarray(t_value, dtype=np.float32))
    n = flow_01.shape[0]

    plan, geom = _derive_plan(flow_01, flow_10, t_value)
    nc = _build_program(plan, geom, n)

    in_maps = []
    for i in range(n):
        in_maps.append({
            "flow01": flow_01[i],
            "flow10": flow_10[i],
            "tv": np.full((P, 1), t_value[i].reshape(()), dtype=np.float32),
        })
    return nc, in_maps, n


def kernel(flow_01, flow_10, t_value):
    from concourse.bass_utils import run_bass_kernel_spmd

    nc, in_maps, n = _prepare(flow_01, flow_10, t_value)
    res = run_bass_kernel_spmd(nc, in_maps, list(range(n)))
    out0 = np.stack([res.results[i]["out0"] for i in range(n)])
    out1 = np.stack([res.results[i]["out1"] for i in range(n)])
    return out0, out1


def _make_runner(nc, in_maps, n_cores):
    """Mirror bass2jax.run_bass_via_pjrt's multi-core path, but return a
    cached jitted callable (no donation) so repeated timed runs are possible."""
    import jax
    from jax.sharding import Mesh, PartitionSpec, NamedSharding
    from jax.experimental.shard_map import shard_map
    from concourse import bass2jax, mybir

    bass2jax.install_neuronx_cc_hook()
    partition_name = nc.partition_id_tensor.name if nc.partition_id_tensor else None
    in_names, out_names, out_avals, zero_outs = [], [], [], []
    for alloc in nc.m.functions[0].allocations:
        if not isinstance(alloc, mybir.MemoryLocationSet):
            continue
        name = alloc.memorylocations[0].name
        if alloc.kind == "ExternalInput":
            if name != partition_name:
                in_names.append(name)
        elif alloc.kind == "ExternalOutput":
            shape = tuple(alloc.tensor_shape)
            dtype = mybir.dt.np(alloc.dtype)
            out_names.append(name)
            out_avals.append(jax.core.ShapedArray(shape, dtype))
            zero_outs.append(np.zeros(shape, dtype))
    n_params = len(in_names)
    all_in_names = in_names + out_names
    if partition_name is not None:
        all_in_names.append(partition_name)

    def _body(*args):
        operands = list(args)
        if partition_name is not None:
            operands.append(bass2jax.partition_id_tensor())
        return tuple(bass2jax._bass_exec_p.bind(
            *operands,
            out_avals=tuple(out_avals),
            in_names=tuple(all_in_names),
            out_names=tuple(out_names),
            lowering_input_output_aliases=(),
            sim_require_finite=True,
            sim_require_nnan=True,
            nc=nc,
        ))

    devices = jax.devices()[:n_cores]
    mesh = Mesh(np.asarray(devices), ("core",))
    in_specs = (PartitionSpec("core"),) * (n_params + len(out_names))
    out_specs = (PartitionSpec("core"),) * len(out_names)
    fn = jax.jit(shard_map(_body, mesh=mesh, in_specs=in_specs,
                           out_specs=out_specs, check_rep=False))
    per_core = [[np.asarray(m[nm]) for nm in in_names] for m in in_maps]
    concat_in = [np.concatenate([per_core[c][i] for c in range(n_cores)], axis=0)
                 for i in range(n_params)]
    concat_zero = [np.concatenate([z] * n_cores, axis=0) for z in zero_outs]
    sh = NamedSharding(mesh, PartitionSpec("core"))
    concat_in = [jax.device_put(a, sh) for a in concat_in]
    concat_zero = [jax.device_put(a, sh) for a in concat_zero]
    return fn, concat_in, concat_zero


def bench(flow_01, flow_10, t_value, iters=8):
    """Wall-clock the jitted SPMD executable; returns min per-iter ns."""
    import time
    import jax

    nc, in_maps, n = _prepare(flow_01, flow_10, t_value)
    fn, concat_in, concat_zero = _make_runner(nc, in_maps, n)
    out = fn(*concat_in, *concat_zero)
    jax.block_until_ready(out)
    times = []
    for _ in range(iters):
        t0 = time.perf_counter()
        out = fn(*concat_in, *concat_zero)
        jax.block_until_ready(out)
        times.append(time.perf_counter() - t0)
    print("bench iters (ms):", [round(t * 1e3, 2) for t in times])
    return int(min(times) * 1e9)
